# revision 4
# baseline (speedup 1.0000x reference)
"""Trainium2 Bass kernel for attention energies + softmax.

Computes: energies = encoder_outputs[8192,4096] @ hidden[4096] ; softmax -> [1,1,8192]

Sharding: encoder_outputs split along seq_len across 8 NeuronCores
(1024 rows each). Each core streams its 16 MiB shard from HBM into
SBUF, computes local energies with fused multiply+accumulate
(scalar_tensor_tensor) on the DVE, and emits the local softmax
numerators n = exp(e - m6) plus its local exp reference m6. The
8-way softmax combine is applied during the host-side gather with
the standard log-sum-exp rescale (out_c = n_c * exp(m_c - M) / S),
which is exact for any finite per-core reference.

Key structure (from perfetto/NTFF analysis on trn2):
- No collectives. The ncfw collective path costs a fixed ~61 us
  firmware-boot barrier + ~11 us cold first-dispatch + ~15 us of
  serialized warmup+AllGather before a 32 B stats exchange can
  complete (measured 97 us total vs 51 us for the last HBM byte).
  Exchanging only per-core (m, s) stats at gather time removes that
  entire tail; the kernel is then HBM-stream-bound end to end.
- The eo stream runs on the sync HWDGE queue at ~396 GB/s. Tiles are
  loaded as 1 MiB halves matching the DVE's stt operand split so the
  DVE (2.29 us per half vs 2.65 us arrival) tracks the stream with no
  full-tile granularity stalls; tile 7 arrives as four 512 KiB
  quarters so the last multiply trails the last HBM byte by ~1.2 us.
- Warmup ops: a DVE memset+stt absorbs the ~2.8 us first-stt ucode
  penalty, and a tiny gpsimd partition_broadcast/all_reduce pair
  absorbs the ~7 us Q7 cold-dispatch, so the h broadcast (2 x 3.25 us
  on Q7) completes before tile 0 lands instead of gating the DVE.
- Local stats use max over tiles 0..6 only (m6) as the exp reference;
  exact math (any per-core reference works in the global combine),
  decoupling the exp chain from the last tile. Tile 7 may exceed m6
  (on this data by up to ~81, giving numerators up to ~1e35 — finite
  in fp32, and the combine math stays exact for any finite values).
- Output is [P, 9]: cols 0..6 the tile 0..6 numerators, col 7 the m6
  reference, col 8 the tile 7 numerators. Cols 0..7 are DMAed out
  while tile 7 still streams; only the 512 B col-8 write trails the
  last multiply. The host transposes each core's [128, 8] numerator
  block (free) and reduces s_c in fp64 from the returned numerators
  (more accurate than an fp32 accum).
"""

from contextlib import ExitStack

import numpy as np

import concourse.bacc as bacc
import concourse.tile as tile
from concourse import bass_isa, mybir
from concourse.bass_utils import run_bass_kernel_spmd

P = 128          # SBUF partitions
H = 4096         # hidden dim
S = 8192         # full seq len
NCORES = 8
SL = S // NCORES  # 1024 rows per core
T = SL // P       # 8 seq tiles per core
HH = H // 2      # half hidden
HQ = H // 4      # quarter hidden

F32 = mybir.dt.float32
AX = mybir.AxisListType
OP = mybir.AluOpType
ACT = mybir.ActivationFunctionType


def build_kernel():
    nc = bacc.Bacc(
        "TRN2",
        target_bir_lowering=False,
        debug=False,
        num_devices=NCORES,
    )
    hidden_d = nc.dram_tensor("hidden", [1, H], F32, kind="ExternalInput").ap()
    eo_d = nc.dram_tensor("eo", [SL, H], F32, kind="ExternalInput").ap()
    out_d = nc.dram_tensor("out", [P, T + 1], F32, kind="ExternalOutput").ap()

    eo_t = eo_d.rearrange("(t p) h -> t p h", p=P)

    with tile.TileContext(nc) as tc, ExitStack() as ctx:
        sb = ctx.enter_context(tc.tile_pool(name="sb", bufs=1))

        # ---- tiles ----
        h_row = sb.tile([1, H], F32)
        h_sbA = sb.tile([P, HH], F32)   # broadcast h[0:2048]
        h_sbB = sb.tile([P, HH], F32)   # broadcast h[2048:4096]
        eo_sb = [
            sb.tile([P, H], F32, name=f"eo{t}") for t in range(T - 1)
        ]
        eo7q = [
            sb.tile([P, HQ], F32, name=f"eo7q{q}") for q in range(4)
        ]
        scrA = sb.tile([P, HH], F32)    # stt dummy out
        eA6 = sb.tile([P, T - 1], F32)  # tiles 0..6, low-H partial dots
        eB6 = sb.tile([P, T - 1], F32)  # tiles 0..6, high-H partial dots
        e7q4 = sb.tile([P, 4], F32)     # tile 7 quarter partial dots
        e06 = sb.tile([P, T - 1], F32)  # energies, tiles 0..6
        e7 = sb.tile([P, 1], F32)       # energies, tile 7
        m_p = sb.tile([P, 1], F32)      # per-partition max, tiles 0..6
        m6_all = sb.tile([P, 1], F32)   # m6 on all partitions
        nmb = sb.tile([P, 1], F32)      # -m6 broadcast to all partitions
        o_sb = sb.tile([P, T + 1], F32)  # cols 0..6 n, col 7 m6, col 8 n7
        # ---- startup ----
        # hidden first on the sync queue (16 KiB; delays eo by ~50 ns),
        # then the eo stream: tiles 0..6 as 1 MiB halves (8 KiB lines),
        # tile 7 as four 512 KiB quarters.
        nc.sync.dma_start(out=h_row[:], in_=hidden_d)
        for t in range(T - 1):
            nc.sync.dma_start(out=eo_sb[t][:], in_=eo_t[t])
        for q in range(4):
            nc.sync.dma_start(
                out=eo7q[q][:], in_=eo_t[T - 1, :, q * HQ : (q + 1) * HQ]
            )

        # h broadcast to 128 partitions on gpsimd (low half first: the
        # DVE consumes it first).
        nc.gpsimd.partition_broadcast(h_sbA[:], h_row[:, 0:HH])
        nc.gpsimd.partition_broadcast(h_sbB[:], h_row[:, HH:H])

        # ---- local energies (fused mult+accum on DVE) ----
        for t in range(T - 1):
            nc.vector.scalar_tensor_tensor(
                out=scrA[:],
                in0=eo_sb[t][:, 0:HH],
                scalar=1.0,
                in1=h_sbA[:],
                op0=OP.mult,
                op1=OP.mult,
                accum_out=eA6[:, t : t + 1],
            )
            nc.vector.scalar_tensor_tensor(
                out=scrA[:],
                in0=eo_sb[t][:, HH:H],
                scalar=1.0,
                in1=h_sbB[:],
                op0=OP.mult,
                op1=OP.mult,
                accum_out=eB6[:, t : t + 1],
            )

        # energies for tiles 0..6 (ready while tile 7 still streams)
        nc.vector.tensor_tensor(out=e06[:], in0=eA6[:], in1=eB6[:], op=OP.add)
        # m6 = max over tiles 0..6 (cross-partition via gpsimd); the exp
        # reference for this core.
        nc.vector.tensor_reduce(out=m_p[:], in_=e06[:], axis=AX.X, op=OP.max)
        nc.gpsimd.partition_all_reduce(
            m6_all[:], m_p[:], channels=P, reduce_op=bass_isa.ReduceOp.max
        )
        nc.scalar.mul(nmb[:], m6_all[:], -1.0)
        nc.scalar.copy(o_sb[:, T - 1 : T], m6_all[:])
        # numerators for tiles 0..6, then ship cols 0..7 while tile 7
        # still streams / computes
        nc.scalar.activation(
            o_sb[:, 0 : T - 1], e06[:], ACT.Exp, bias=nmb[:], scale=1.0
        )

        # tile 7 quarters on DVE (short tail after the last HBM byte)
        for q in range(4):
            h_half = h_sbA if q < 2 else h_sbB
            hoff = (q % 2) * HQ
            nc.vector.scalar_tensor_tensor(
                out=scrA[:, 0:HQ],
                in0=eo7q[q][:],
                scalar=1.0,
                in1=h_half[:, hoff : hoff + HQ],
                op0=OP.mult,
                op1=OP.mult,
                accum_out=e7q4[:, q : q + 1],
            )
        nc.vector.tensor_reduce(out=e7[:], in_=e7q4[:], axis=AX.X, op=OP.add)
        nc.scalar.activation(
            o_sb[:, T : T + 1], e7[:], ACT.Exp, bias=nmb[:], scale=1.0
        )
        nc.scalar.dma_start(out=out_d, in_=o_sb[:])

    nc.compile()
    return nc


_NC = None


def _get_nc():
    global _NC
    if _NC is None:
        _NC = build_kernel()
    return _NC


def _make_in_maps(hidden: np.ndarray, encoder_outputs: np.ndarray):
    hidden = np.ascontiguousarray(np.asarray(hidden, dtype=np.float32)).reshape(1, H)
    eo = np.ascontiguousarray(np.asarray(encoder_outputs, dtype=np.float32))
    assert eo.shape == (S, H), eo.shape
    return [
        {"hidden": hidden, "eo": eo[c * SL : (c + 1) * SL]} for c in range(NCORES)
    ]


def _combine(bufs) -> np.ndarray:
    """Host-side softmax combine of the 8 shards (exact log-sum-exp).

    bufs[c] is the core-c [P, T+1] output: cols 0..T-2 the tile 0..6
    numerators n = exp(e - m_c), col T-1 the reference m_c, col T the
    tile 7 numerators.
    """
    n = np.empty((NCORES, SL), dtype=np.float64)
    m = np.empty(NCORES, dtype=np.float64)
    for c, buf in enumerate(bufs):
        b = np.asarray(buf, dtype=np.float64).reshape(P, T + 1)
        n[c, : SL - P] = b[:, : T - 1].T.reshape(SL - P)
        n[c, SL - P :] = b[:, T]
        m[c] = b[0, T - 1]
    M = m.max()
    w = np.exp(m - M)                      # per-core rescale to the global ref
    Ssum = (n.sum(axis=1) * w).sum()       # S = sum_c s_c * exp(m_c - M)
    out = n * (w / Ssum)[:, None]
    return out.reshape(1, 1, S).astype(np.float32)


def kernel(hidden: np.ndarray, encoder_outputs: np.ndarray) -> np.ndarray:
    nc = _get_nc()
    in_maps = _make_in_maps(hidden, encoder_outputs)
    res = run_bass_kernel_spmd(nc, in_maps, core_ids=list(range(NCORES)))
    return _combine([res.results[c]["out"] for c in range(NCORES)])


if __name__ == "__main__":
    rng = np.random.default_rng(0)
    h = rng.standard_normal((1, H), dtype=np.float32)
    eo = rng.standard_normal((S, H), dtype=np.float32)
    got = kernel(hidden=h, encoder_outputs=eo)
    e = eo.astype(np.float64) @ h.reshape(-1).astype(np.float64)
    e -= e.max()
    p = np.exp(e)
    want = (p / p.sum()).reshape(1, 1, S)
    err = np.abs(got.astype(np.float64) - want)
    rel = err.max() / np.abs(want).max()
    print("max abs err:", err.max(), "rel:", rel)


# revision 6
# speedup vs baseline: 1.0727x; 1.0727x over previous
"""Trainium2 Bass kernel for attention energies + softmax.

Computes: energies = encoder_outputs[8192,4096] @ hidden[4096] ; softmax -> [1,1,8192]

Sharding: encoder_outputs split along seq_len across 8 NeuronCores
(1024 rows each). Each core streams its 16 MiB shard from HBM into
SBUF, computes local energies with fused multiply+accumulate
(scalar_tensor_tensor) on the DVE, and emits softmax numerators
n = exp(e - m) with a PER-PARTITION reference m[p] = max_t e[p,t]
(so every numerator is <= 1; no overflow is possible for any data).
The 1024-way softmax combine is applied during the host-side gather
with the standard log-sum-exp rescale
(out[c,p,t] = n[c,p,t] * exp(m[c,p] - M) / S), which is exact for
any finite per-(core,partition) reference.

Key structure (from perfetto/NTFF analysis on trn2):
- No collectives. The ncfw collective path costs a fixed ~61 us
  firmware-boot barrier + ~11 us cold first-dispatch + ~15 us of
  serialized warmup+AllGather before a 32 B stats exchange can
  complete (measured 97 us total vs ~51 us for the last HBM byte).
  Exchanging per-core stats at gather time removes that entire tail;
  the kernel is then HBM-stream-bound end to end.
- No gpsimd. The Q7 cores pay a ~6 us IRAM ucode load on first use
  of each custom op (partition_broadcast/all_reduce), which gated the
  DVE start at ~19 us in earlier revisions. The h broadcast to 128
  partitions is instead done by the DMA engines (stride-0
  partition-broadcast AP from DRAM) on the scalar HWDGE ring, in
  parallel with the eo stream on the sync ring; the per-partition
  exp reference needs no cross-partition reduction at all.
- The eo stream runs at ~340-400 GB/s (run-to-run HBM variance).
  Tiles 0..6 are loaded as 1 MiB halves matching the DVE's stt
  operand split (2.29 us consume vs ~2.7 us arrival per half) so the
  DVE tracks the stream with no full-tile granularity stalls; tile 7
  arrives as four 512 KiB quarters so the last multiply trails the
  last HBM byte by ~1.2 us.
- A DVE memset+stt warmup absorbs the ~2.8 us first-stt penalty.
- Output is [P, 9]: cols 0..7 the numerators for tiles 0..7, col 8
  the per-partition reference m. One 4.6 KiB DMA on the scalar ring.
  The host transposes each core's [128, 8] numerator block (free)
  and reduces s_c in fp64 from the returned numerators.
"""

from contextlib import ExitStack

import numpy as np

import concourse.bacc as bacc
import concourse.tile as tile
from concourse import mybir
from concourse.bass_utils import run_bass_kernel_spmd

P = 128          # SBUF partitions
H = 4096         # hidden dim
S = 8192         # full seq len
NCORES = 8
SL = S // NCORES  # 1024 rows per core
T = SL // P       # 8 seq tiles per core
HH = H // 2      # half hidden
HQ = H // 4      # quarter hidden

F32 = mybir.dt.float32
AX = mybir.AxisListType
OP = mybir.AluOpType
ACT = mybir.ActivationFunctionType


def build_kernel():
    nc = bacc.Bacc(
        "TRN2",
        target_bir_lowering=False,
        debug=False,
        num_devices=NCORES,
    )
    hidden_d = nc.dram_tensor("hidden", [1, H], F32, kind="ExternalInput").ap()
    eo_d = nc.dram_tensor("eo", [SL, H], F32, kind="ExternalInput").ap()
    out_d = nc.dram_tensor("out", [P, T + 1], F32, kind="ExternalOutput").ap()

    eo_t = eo_d.rearrange("(t p) h -> t p h", p=P)

    with tile.TileContext(nc) as tc, ExitStack() as ctx:
        sb = ctx.enter_context(tc.tile_pool(name="sb", bufs=1))

        # ---- tiles ----
        h_sbA = sb.tile([P, HH], F32)   # h[0:2048] on all partitions
        h_sbB = sb.tile([P, HH], F32)   # h[2048:4096] on all partitions
        eo_sb = [
            sb.tile([P, H], F32, name=f"eo{t}") for t in range(T - 1)
        ]
        eo7q = [
            sb.tile([P, HQ], F32, name=f"eo7q{q}") for q in range(4)
        ]
        scrA = sb.tile([P, HH], F32)    # stt dummy out
        eA6 = sb.tile([P, T - 1], F32)  # tiles 0..6, low-H partial dots
        eB6 = sb.tile([P, T - 1], F32)  # tiles 0..6, high-H partial dots
        e7q4 = sb.tile([P, 4], F32)     # tile 7 quarter partial dots
        e_all = sb.tile([P, T], F32)    # energies, all 8 tiles
        m_p = sb.tile([P, 1], F32)      # per-partition max (exp reference)
        nmb = sb.tile([P, 1], F32)      # -m_p
        o_sb = sb.tile([P, T + 1], F32)  # cols 0..7 numerators, col 8 m_p
        wrm = sb.tile([P, 8], F32)      # DVE warmup scratch
        wrm2 = sb.tile([P, 8], F32)
        wacc = sb.tile([P, 1], F32)

        # ---- warmups (absorb DVE first-stt ucode penalty) ----
        nc.vector.memset(wrm[:], 0.0)
        nc.vector.scalar_tensor_tensor(
            out=wrm2[:], in0=wrm[:], scalar=1.0, in1=wrm[:],
            op0=OP.mult, op1=OP.mult, accum_out=wacc[:],
        )

        # ---- startup ----
        # h replicated to all 128 partitions by the DMA engines on the
        # scalar HWDGE ring (2 x 1 MiB, same-address DRAM reads),
        # concurrent with the eo stream on the sync ring.
        nc.scalar.dma_start(
            out=h_sbA[:], in_=hidden_d[:, 0:HH].partition_broadcast(P)
        )
        nc.scalar.dma_start(
            out=h_sbB[:], in_=hidden_d[:, HH:H].partition_broadcast(P)
        )
        # eo stream: tiles 0..6 as 1 MiB halves (8 KiB lines), tile 7
        # as four 512 KiB quarters.
        for t in range(T - 1):
            nc.sync.dma_start(out=eo_sb[t][:, 0:HH], in_=eo_t[t, :, 0:HH])
            nc.sync.dma_start(out=eo_sb[t][:, HH:H], in_=eo_t[t, :, HH:H])
        for q in range(4):
            nc.sync.dma_start(
                out=eo7q[q][:], in_=eo_t[T - 1, :, q * HQ : (q + 1) * HQ]
            )

        # ---- local energies (fused mult+accum on DVE) ----
        for t in range(T - 1):
            nc.vector.scalar_tensor_tensor(
                out=scrA[:],
                in0=eo_sb[t][:, 0:HH],
                scalar=1.0,
                in1=h_sbA[:],
                op0=OP.mult,
                op1=OP.mult,
                accum_out=eA6[:, t : t + 1],
            )
            nc.vector.scalar_tensor_tensor(
                out=scrA[:],
                in0=eo_sb[t][:, HH:H],
                scalar=1.0,
                in1=h_sbB[:],
                op0=OP.mult,
                op1=OP.mult,
                accum_out=eB6[:, t : t + 1],
            )

        # energies for tiles 0..6 (ready while tile 7 still streams)
        nc.vector.tensor_tensor(
            out=e_all[:, 0 : T - 1], in0=eA6[:], in1=eB6[:], op=OP.add
        )

        # tile 7 quarters on DVE (short tail after the last HBM byte)
        for q in range(4):
            h_half = h_sbA if q < 2 else h_sbB
            hoff = (q % 2) * HQ
            nc.vector.scalar_tensor_tensor(
                out=scrA[:, 0:HQ],
                in0=eo7q[q][:],
                scalar=1.0,
                in1=h_half[:, hoff : hoff + HQ],
                op0=OP.mult,
                op1=OP.mult,
                accum_out=e7q4[:, q : q + 1],
            )
        nc.vector.tensor_reduce(
            out=e_all[:, T - 1 : T], in_=e7q4[:], axis=AX.X, op=OP.add
        )
        # per-partition reference over all 8 tiles -> numerators <= 1
        nc.vector.tensor_reduce(out=m_p[:], in_=e_all[:], axis=AX.X, op=OP.max)
        nc.vector.tensor_scalar_mul(nmb[:], m_p[:], -1.0)
        nc.scalar.copy(o_sb[:, T : T + 1], m_p[:])
        nc.scalar.activation(
            o_sb[:, 0:T], e_all[:], ACT.Exp, bias=nmb[:], scale=1.0
        )
        nc.scalar.dma_start(out=out_d, in_=o_sb[:])

    nc.compile()
    return nc


_NC = None


def _get_nc():
    global _NC
    if _NC is None:
        _NC = build_kernel()
    return _NC


def _make_in_maps(hidden: np.ndarray, encoder_outputs: np.ndarray):
    hidden = np.ascontiguousarray(np.asarray(hidden, dtype=np.float32)).reshape(1, H)
    eo = np.ascontiguousarray(np.asarray(encoder_outputs, dtype=np.float32))
    assert eo.shape == (S, H), eo.shape
    return [
        {"hidden": hidden, "eo": eo[c * SL : (c + 1) * SL]} for c in range(NCORES)
    ]


def _combine(bufs) -> np.ndarray:
    """Host-side softmax combine of the 8x128 shards (exact log-sum-exp).

    bufs[c] is the core-c [P, T+1] output: cols 0..T-1 the numerators
    n[p,t] = exp(e[p,t] - m[p]), col T the per-partition reference m[p].
    """
    n = np.empty((NCORES, P, T), dtype=np.float64)
    m = np.empty((NCORES, P), dtype=np.float64)
    for c, buf in enumerate(bufs):
        b = np.asarray(buf, dtype=np.float64).reshape(P, T + 1)
        n[c] = b[:, :T]
        m[c] = b[:, T]
    M = m.max()
    w = np.exp(m - M)                      # per-(core,partition) rescale
    Ssum = (n.sum(axis=2) * w).sum()       # S = sum_cp s_cp * exp(m_cp - M)
    out = n * (w / Ssum)[:, :, None]
    # seq index within a core is t*P + p -> transpose (p, t) -> (t, p)
    return out.transpose(0, 2, 1).reshape(1, 1, S).astype(np.float32)


def kernel(hidden: np.ndarray, encoder_outputs: np.ndarray) -> np.ndarray:
    nc = _get_nc()
    in_maps = _make_in_maps(hidden, encoder_outputs)
    res = run_bass_kernel_spmd(nc, in_maps, core_ids=list(range(NCORES)))
    return _combine([res.results[c]["out"] for c in range(NCORES)])


if __name__ == "__main__":
    rng = np.random.default_rng(0)
    h = rng.standard_normal((1, H), dtype=np.float32)
    eo = rng.standard_normal((S, H), dtype=np.float32)
    got = kernel(hidden=h, encoder_outputs=eo)
    e = eo.astype(np.float64) @ h.reshape(-1).astype(np.float64)
    e -= e.max()
    p = np.exp(e)
    want = (p / p.sum()).reshape(1, 1, S)
    err = np.abs(got.astype(np.float64) - want)
    rel = err.max() / np.abs(want).max()
    print("max abs err:", err.max(), "rel:", rel)
